# revision 5
# baseline (speedup 1.0000x reference)
"""Trainium2 Bass kernel for ConvexContractionAttention.

Math notes (derived from the reference):
  block(xi, w, b, a, g, beta) with h = xi*softplus(w)+b, h' = h @ qr(a).Q,
  then batch-norm over (B,T) per (d,j) feature reduces to an affine map of
  the centered input channel:
      out[b,t,d,j] = (xi[b,t,d] - mean_d(xi)) * A[d,j] + beta[d,j]
      A[d,j] = u[d,j]*g[d,j] / sqrt(var_d(xi)*u[d,j]^2 + eps_norm)
      u = softplus(w) @ Q          (bias b cancels through the mean)
  With beta == 0 (true for setup_inputs), per channel d:
      p    = xcq * xck
      s_j  = sigmoid(gamma*Aq_j*Ak_j * p)
      out0 = xcv * (sum_j s_j*Av_j) / (sum_j s_j + eps_w)
  followed by one more batch-affine-norm over (B,T) per channel.

Sharding: channel dim d=1024 split 128-per-core across 8 cores (fully
independent per channel; no collectives). On-chip layout: channels on the
128 SBUF partitions, B*T=8192 on the free axis; the host pre-transposes
each core's shard so every DMA is contiguous.
"""

import sys

if "/opt/trn_rl_repo" not in sys.path:
    sys.path.insert(0, "/opt/trn_rl_repo")

import numpy as np

import concourse.bacc as bacc
import concourse.tile as tile
from concourse import mybir
from concourse import bass_utils

B, T, D = 4, 2048, 1024
BT = B * T
N_CORES = 8
DL = D // N_CORES  # 128 channels per core == SBUF partitions
GAMMA = 5.0
EPS_NORM = 1e-5
EPS_W = 1e-8

F32 = mybir.dt.float32
Act = mybir.ActivationFunctionType
Alu = mybir.AluOpType


def _emit_body(nc, tc, dram, F_main=1024, F_fin=2048):
    """Emit one full pass (phases 1-4) of the per-core kernel."""
    import contextlib

    with contextlib.ExitStack() as ctx:
        resident = ctx.enter_context(tc.tile_pool(name="resident", bufs=1))
        consts = ctx.enter_context(tc.tile_pool(name="consts", bufs=1))
        temps = ctx.enter_context(tc.tile_pool(name="temps", bufs=2))
        stage = ctx.enter_context(tc.tile_pool(name="stage", bufs=3))

        chunks = ("q", "k", "v")

        # ---- Phase 1: load x shards + per-channel mean/var via bn_stats ----
        x_sb = {}
        mv = {}
        NSUB = BT // 512
        for p in chunks:
            x_sb[p] = resident.tile([DL, BT], F32, name=f"x_{p}", tag=f"x_{p}")
            for i in range(4):
                sl = slice(i * (BT // 4), (i + 1) * (BT // 4))
                nc.sync.dma_start(out=x_sb[p][:, sl], in_=dram["x" + p][:, sl])
            st = temps.tile([DL, NSUB, 6], F32, name="bnst", tag="bnst")
            for i in range(NSUB):
                nc.vector.bn_stats(
                    out=st[:, i, :], in_=x_sb[p][:, i * 512 : (i + 1) * 512]
                )
            mv[p] = consts.tile([DL, 2], F32, name=f"mv_{p}", tag=f"mv_{p}")
            nc.vector.bn_aggr(out=mv[p], in_=st)

        # ---- Phase 1b: per-partition constants ----
        u_sb, ug_sb, Amat = {}, {}, {}
        for p in chunks:
            u_sb[p] = consts.tile([DL, 3], F32, name=f"u_{p}", tag=f"u_{p}")
            ug_sb[p] = consts.tile([DL, 3], F32, name=f"ug_{p}", tag=f"ug_{p}")
            nc.sync.dma_start(out=u_sb[p], in_=dram["u" + p])
            nc.sync.dma_start(out=ug_sb[p], in_=dram["ug" + p])
        g_out_sb = consts.tile([DL, 1], F32, name="g_out", tag="g_out")
        b_out_sb = consts.tile([DL, 1], F32, name="b_out", tag="b_out")
        nc.sync.dma_start(out=g_out_sb, in_=dram["g_out"])
        nc.sync.dma_start(out=b_out_sb, in_=dram["b_out"])

        neg_mu = {}
        for p in chunks:
            var = mv[p][:, 1:2]
            neg_mu[p] = consts.tile([DL, 1], F32, name=f"negmu_{p}", tag=f"negmu_{p}")
            nc.vector.tensor_scalar_mul(out=neg_mu[p], in0=mv[p][:, 0:1], scalar1=-1.0)
            usq = temps.tile([DL, 3], F32, name="usq", tag="usq")
            nc.vector.tensor_mul(usq, u_sb[p], u_sb[p])
            vterm = temps.tile([DL, 3], F32, name="vterm", tag="vterm")
            nc.vector.tensor_scalar(
                out=vterm, in0=usq, scalar1=var, scalar2=EPS_NORM,
                op0=Alu.mult, op1=Alu.add,
            )
            sd = temps.tile([DL, 3], F32, name="sd3", tag="sd3")
            nc.scalar.sqrt(sd, vterm)
            inv = temps.tile([DL, 3], F32, name="inv3", tag="inv3")
            nc.vector.reciprocal(out=inv, in_=sd)
            Amat[p] = consts.tile([DL, 3], F32, name=f"A_{p}", tag=f"A_{p}")
            nc.vector.tensor_mul(Amat[p], ug_sb[p], inv)

        cmat = consts.tile([DL, 3], F32, name="cmat", tag="cmat")
        nc.vector.tensor_mul(cmat, Amat["q"], Amat["k"])
        nc.vector.tensor_scalar_mul(out=cmat, in0=cmat, scalar1=GAMMA)

        # ---- Phase 2: main loop ----
        out_un = resident.tile([DL, BT], F32, name="out_un", tag="out_un")
        muq = mv["q"][:, 0:1]
        muv = mv["v"][:, 0:1]
        Av = Amat["v"]
        for i in range(BT // F_main):
            sl = slice(i * F_main, (i + 1) * F_main)
            xq_s, xk_s, xv_s = x_sb["q"][:, sl], x_sb["k"][:, sl], x_sb["v"][:, sl]

            bA = temps.tile([DL, F_main], F32, name="bA", tag="bA")
            bB = temps.tile([DL, F_main], F32, name="bB", tag="bB")
            bC = temps.tile([DL, F_main], F32, name="bC", tag="bC")
            bD = temps.tile([DL, F_main], F32, name="bD", tag="bD")
            bE = temps.tile([DL, F_main], F32, name="bE", tag="bE")
            bF = temps.tile([DL, F_main], F32, name="bF", tag="bF")

            # xck = xk - mu_k    (ScalarE)
            nc.scalar.activation(bA, xk_s, Act.Identity, bias=neg_mu["k"], scale=1.0)
            # p = (xq - mu_q) * xck
            nc.vector.scalar_tensor_tensor(
                out=bB, in0=xq_s, scalar=muq, in1=bA,
                op0=Alu.subtract, op1=Alu.mult,
            )
            # s_j = sigmoid(c_j * p)
            nc.scalar.activation(bC, bB, Act.Sigmoid, scale=cmat[:, 0:1])
            nc.scalar.activation(bD, bB, Act.Sigmoid, scale=cmat[:, 1:2])
            # t_j = Av_j * s_j
            nc.scalar.activation(bE, bC, Act.Copy, scale=Av[:, 0:1])
            nc.scalar.activation(bF, bD, Act.Copy, scale=Av[:, 1:2])
            # den01 = s0 + s1  (into bC, in-place)
            nc.vector.tensor_add(bC, bC, bD)
            # s2 = sigmoid(c2 * p)  (into bB, in-place over p)
            nc.scalar.activation(bB, bB, Act.Sigmoid, scale=cmat[:, 2:3])
            # t2 = Av2 * s2
            nc.scalar.activation(bA, bB, Act.Copy, scale=Av[:, 2:3])
            # den = (s2 + eps_w) + den01
            nc.vector.scalar_tensor_tensor(
                out=bD, in0=bB, scalar=EPS_W, in1=bC, op0=Alu.add, op1=Alu.add
            )
            # r = 1/den
            nc.vector.reciprocal_approx_fast(out=bB, in_=bD)
            # num = t0 + t1 + t2 ; tt = num * r
            nc.vector.tensor_add(bE, bE, bF)
            nc.vector.tensor_add(bE, bE, bA)
            nc.vector.tensor_mul(bE, bE, bB)
            # out_un = (xv - mu_v) * tt
            nc.vector.scalar_tensor_tensor(
                out=out_un[:, sl], in0=xv_s, scalar=muv, in1=bE,
                op0=Alu.subtract, op1=Alu.mult,
            )

        # ---- Phase 3: final norm constants ----
        st_o = temps.tile([DL, NSUB, 6], F32, name="bnst_o", tag="bnst_o")
        for i in range(NSUB):
            nc.vector.bn_stats(out=st_o[:, i, :], in_=out_un[:, i * 512 : (i + 1) * 512])
        mv_o = consts.tile([DL, 2], F32, name="mv_o", tag="mv_o")
        nc.vector.bn_aggr(out=mv_o, in_=st_o)
        eps_t = consts.tile([DL, 1], F32, name="eps_t", tag="eps_t")
        nc.vector.memset(eps_t, EPS_NORM)
        sd_o = consts.tile([DL, 1], F32, name="sd_o", tag="sd_o")
        nc.scalar.activation(sd_o, mv_o[:, 1:2], Act.Sqrt, bias=eps_t, scale=1.0)
        rs_o = consts.tile([DL, 1], F32, name="rs_o", tag="rs_o")
        nc.vector.reciprocal(out=rs_o, in_=sd_o)
        fs = consts.tile([DL, 1], F32, name="fs", tag="fs")
        nc.vector.tensor_mul(fs, g_out_sb, rs_o)
        fbt = consts.tile([DL, 1], F32, name="fbt", tag="fbt")
        nc.vector.tensor_mul(fbt, mv_o[:, 0:1], fs)
        fb = consts.tile([DL, 1], F32, name="fb", tag="fb")
        nc.vector.tensor_sub(fb, b_out_sb, fbt)

        # ---- Phase 4: final affine + store ----
        for i in range(BT // F_fin):
            sl = slice(i * F_fin, (i + 1) * F_fin)
            stg = stage.tile([DL, F_fin], F32, name="stg", tag="stg")
            nc.scalar.activation(stg, out_un[:, sl], Act.Identity, bias=fb, scale=fs)
            nc.sync.dma_start(out=dram["out"][:, sl], in_=stg)


def build_program(reps=1):
    nc = bacc.Bacc("TRN2", num_devices=N_CORES)
    dram = {}
    for p in ("q", "k", "v"):
        dram["x" + p] = nc.dram_tensor("x" + p, [DL, BT], F32, kind="ExternalInput").ap()
        dram["u" + p] = nc.dram_tensor("u" + p, [DL, 3], F32, kind="ExternalInput").ap()
        dram["ug" + p] = nc.dram_tensor("ug" + p, [DL, 3], F32, kind="ExternalInput").ap()
    dram["g_out"] = nc.dram_tensor("g_out", [DL, 1], F32, kind="ExternalInput").ap()
    dram["b_out"] = nc.dram_tensor("b_out", [DL, 1], F32, kind="ExternalInput").ap()
    dram["out"] = nc.dram_tensor("out", [DL, BT], F32, kind="ExternalOutput").ap()

    with tile.TileContext(nc) as tc:
        for _ in range(reps):
            _emit_body(nc, tc, dram)
    nc.compile()
    return nc


def _softplus(x):
    return np.log1p(np.exp(-np.abs(x))) + np.maximum(x, 0.0)


def _host_params(w, b, a, g, beta):
    """Return (u, u*g) per channel; None if beta!=0 (fast path invalid)."""
    Q = np.linalg.qr(np.asarray(a, dtype=np.float64))[0].astype(np.float32)
    u = np.einsum("di,dij->dj", _softplus(np.asarray(w, np.float64)).astype(np.float32), Q)
    return u, u * np.asarray(g, np.float32)


def _reference_fallback(x, wq, bq, aq, gq, betaq, wk, bk, ak, gk, betak,
                        wv, bv, av, gv, betav, g_out, b_out):
    """General-path numpy fallback (only used if some beta is nonzero)."""
    def block(xi, w, b, a, g, beta):
        h = xi[..., None] * _softplus(w) + b
        Q = np.linalg.qr(a)[0]
        h = np.einsum("btdi,dij->btdj", h, Q)
        mean = h.mean(axis=(0, 1))
        var = h.var(axis=(0, 1))
        return (h - mean) / np.sqrt(var + EPS_NORM) * g + beta

    d = D
    Qp = block(x[..., :d], wq, bq, aq, gq, betaq)
    Kp = block(x[..., d:2 * d], wk, bk, ak, gk, betak)
    Vp = block(x[..., 2 * d:], wv, bv, av, gv, betav)
    scores = 1.0 / (1.0 + np.exp(-GAMMA * (Qp * Kp)))
    weights = scores / (scores.sum(axis=-1, keepdims=True) + EPS_W)
    out = (weights * Vp).sum(axis=-1)
    mean = out.mean(axis=(0, 1))
    var = out.var(axis=(0, 1))
    return ((out - mean) / np.sqrt(var + EPS_NORM) * g_out + b_out).astype(np.float32)


_NC_CACHE = {}


def _get_program(reps=1):
    if reps not in _NC_CACHE:
        _NC_CACHE[reps] = build_program(reps)
    return _NC_CACHE[reps]


def _make_in_maps(x, params):
    """params: dict p -> (u, ug) full (D,3); x: (B,T,3D). Returns per-core maps."""
    x2 = np.asarray(x, np.float32).reshape(BT, 3 * D)
    in_maps = []
    for c in range(N_CORES):
        m = {}
        for pi, p in enumerate(("q", "k", "v")):
            cols = slice(pi * D + c * DL, pi * D + (c + 1) * DL)
            m["x" + p] = np.ascontiguousarray(x2[:, cols].T)
            u, ug = params[p]
            m["u" + p] = np.ascontiguousarray(u[c * DL:(c + 1) * DL])
            m["ug" + p] = np.ascontiguousarray(ug[c * DL:(c + 1) * DL])
        m["g_out"] = np.ascontiguousarray(params["g_out"][c * DL:(c + 1) * DL, None])
        m["b_out"] = np.ascontiguousarray(params["b_out"][c * DL:(c + 1) * DL, None])
        in_maps.append(m)
    return in_maps


def kernel(x, wq, bq, aq, gq, betaq, wk, bk, ak, gk, betak,
           wv, bv, av, gv, betav, g_out, b_out):
    if (np.any(np.asarray(betaq)) or np.any(np.asarray(betak))
            or np.any(np.asarray(betav))):
        return _reference_fallback(x, wq, bq, aq, gq, betaq, wk, bk, ak, gk,
                                   betak, wv, bv, av, gv, betav, g_out, b_out)

    params = {
        "q": _host_params(wq, bq, aq, gq, betaq),
        "k": _host_params(wk, bk, ak, gk, betak),
        "v": _host_params(wv, bv, av, gv, betav),
        "g_out": np.asarray(g_out, np.float32),
        "b_out": np.asarray(b_out, np.float32),
    }
    nc = _get_program()
    in_maps = _make_in_maps(x, params)
    res = bass_utils.run_bass_kernel_spmd(nc, in_maps, core_ids=list(range(N_CORES)))
    out = np.empty((BT, D), np.float32)
    for c in range(N_CORES):
        out[:, c * DL:(c + 1) * DL] = res.results[c]["out"].T
    return out.reshape(B, T, D)
